# revision 1
# baseline (speedup 1.0000x reference)
"""MoE (MiMoV2 FlashMoE) Trainium2 kernel: expert-parallel over 8 NeuronCores.

Strategy:
  Phase 1 (device): router — logits = x @ w_router.T computed in fp32,
    top-4 selection via exact max/mask iterations on logits, combine
    weights = sigmoid(logit) normalized over the selected 4. Each core
    handles T/8 = 512 tokens. Output: dense combine matrix [T, E]
    (4 nonzeros per row).
  Host: compaction — per-expert token index lists from combine > 0
    (pure data movement), gather token columns into per-expert capacity-C
    buffers laid out exactly as the SBUF tiles (contiguous DMA).
  Phase 2 (device): experts — 4 experts per core. For each expert:
    G^T = Wg @ Xg^T, U^T = Wu @ Xg^T, Hm = silu(G)*U*combine,
    Y^T = Wd @ Hm. Matmuls in float32r (TF32-like, full PE rate,
    ~1.5e-4 relative error). Output y^T [H, C] per expert, weighted.
  Host: scatter-add per-expert outputs into y [T, H] (unique indices
    per expert, ascending expert order matches reference accumulation).
"""
import math
import numpy as np
from contextlib import ExitStack

import concourse.bass as bass
import concourse.mybir as mybir
import concourse.tile as tile
from concourse import bacc
from concourse.bass_utils import run_bass_kernel_spmd

F32 = mybir.dt.float32
F32R = mybir.dt.float32r

# Problem shapes (hardcoded per contract)
E = 32          # experts
TOPK = 4
H = 1024        # hidden
I = 768         # intermediate
B, S = 2, 2048
T = B * S       # 4096 tokens
NCORES = 8
EPC = E // NCORES    # experts per core = 4
TPC = T // NCORES    # router tokens per core = 512
KH = H // 128        # 8 contraction chunks over H
KI = I // 128        # 6 contraction chunks over I

_program_cache = {}


def _ctiles(C):
    """Split C into near-equal tiles, each <= 512 (PSUM bank) and >= 256
    (float32r full-rate threshold) whenever C >= 512."""
    n = max(1, math.ceil(C / 512))
    base = C // n
    rem = C - base * n
    sizes = [base + (1 if i < rem else 0) for i in range(n)]
    out, off = [], 0
    for s in sizes:
        out.append((off, s))
        off += s
    return out


def build_router(reps=1):
    """Per-core: logits^T = w_router @ x^T via PE (weights stationary, 512
    tokens moving), DVE 32x32 block transposes to [tokens, E], then a
    batched top-4 + combine-weight computation on a single [128, 4, E]
    tile. Selection compares exact fp32 logits."""
    nc = bacc.Bacc()
    NT = TPC // 128  # 4 token tiles
    xTc = nc.dram_tensor("xTc", [H, TPC], F32, kind="ExternalInput")
    wrT = nc.dram_tensor("wrT", [H, E], F32, kind="ExternalInput")
    comb_out = nc.dram_tensor("comb", [NT, 128, E], F32, kind="ExternalOutput")
    with ExitStack() as ctx:
        tc = ctx.enter_context(tile.TileContext(nc))
        sb = ctx.enter_context(tc.tile_pool(name="sb", bufs=1))
        work = ctx.enter_context(tc.tile_pool(name="work", bufs=2))
        ps = ctx.enter_context(tc.tile_pool(name="ps", bufs=2, space="PSUM"))

        xr = sb.tile([128, KH, TPC], F32)
        wr = sb.tile([128, KH, E], F32)
        for k in range(KH):
            nc.sync.dma_start(out=xr[:, k, :], in_=xTc[k * 128:(k + 1) * 128, :])
            nc.sync.dma_start(out=wr[:, k, :], in_=wrT[k * 128:(k + 1) * 128, :])

        for _ in range(reps):
            lgT_p = ps.tile([E, TPC], F32)   # logits^T, 1 PSUM bank
            for k in range(KH):
                nc.tensor.matmul(lgT_p, wr[:, k, :], xr[:, k, :],
                                 start=(k == 0), stop=(k == KH - 1))
            lgT = work.tile([E, TPC], F32)
            nc.vector.tensor_copy(lgT, lgT_p)
            # transpose to [128, NT, E] with DVE 32x32 block transposes
            lt = work.tile([128, NT, E], F32)
            for t in range(NT):
                for jb in range(128 // 32):
                    nc.vector.transpose(
                        lt[jb * 32:(jb + 1) * 32, t, :],
                        lgT[:, t * 128 + jb * 32: t * 128 + (jb + 1) * 32])
            # batched top-4: find 4th max per token via iterative masking
            cur = work.tile([128, NT, E], F32)
            nc.vector.tensor_copy(cur, lt)
            m = work.tile([128, NT, 1], F32)
            ge = work.tile([128, NT, E], F32)
            for _k in range(TOPK - 1):
                nc.vector.reduce_max(m, cur, axis=mybir.AxisListType.X)
                nc.vector.tensor_tensor(ge, cur, m.broadcast_to((128, NT, E)),
                                        op=mybir.AluOpType.is_ge)
                nc.vector.scalar_tensor_tensor(cur, ge, -1e30, cur,
                                               op0=mybir.AluOpType.mult,
                                               op1=mybir.AluOpType.add)
            nc.vector.reduce_max(m, cur, axis=mybir.AxisListType.X)
            # sel = (logits >= 4th max), combine = sel*sigmoid normalized
            sel = work.tile([128, NT, E], F32)
            nc.vector.tensor_tensor(sel, lt, m.broadcast_to((128, NT, E)),
                                    op=mybir.AluOpType.is_ge)
            sig = work.tile([128, NT, E], F32)
            nc.scalar.activation(sig, lt, mybir.ActivationFunctionType.Sigmoid)
            wsel = work.tile([128, NT, E], F32)
            nc.vector.tensor_mul(wsel, sel, sig)
            ssum = work.tile([128, NT, 1], F32)
            nc.vector.reduce_sum(ssum, wsel, axis=mybir.AxisListType.X)
            nc.vector.tensor_scalar_add(ssum, ssum, 1e-20)
            rsum = work.tile([128, NT, 1], F32)
            nc.vector.reciprocal(rsum, ssum)
            ct = work.tile([128, NT, E], F32)
            nc.vector.tensor_tensor(ct, wsel, rsum.broadcast_to((128, NT, E)),
                                    op=mybir.AluOpType.mult)
            for t in range(NT):
                nc.sync.dma_start(out=comb_out[t], in_=ct[:, t, :])
    nc.finalize()
    return nc


def build_experts(C, reps=1):
    """Expert MLP kernel. Per-core inputs (pre-laid-out for SBUF tiles):
      xg  [EPC, 128, KH, C]        f32r  xg[j,p,k,c] = x[tok_c, k*128+p]
      wgu [EPC, KI, 128, KH, 2, 128] f32r  [...,0,i]=w_gate[e,m*128+i,k*128+p]
      wd  [EPC, KH, 128, KI, 128]  f32r  wd[j,h,p,k,o]=w_down[e,h*128+o,k*128+p]
      cw  [EPC, C]                 f32   combine weights (0 on padding)
    Output: yg [EPC, 128, KH, C] f32, yg[j,p,h,c] = y^T[h*128+p, c]
    (combine-weighted, transposed)."""
    nc = bacc.Bacc()
    xg = nc.dram_tensor("xg", [EPC, 128, KH, C], F32R, kind="ExternalInput")
    wgu = nc.dram_tensor("wgu", [EPC, KI, 2, 128, KH, 128], F32R,
                         kind="ExternalInput")
    wd = nc.dram_tensor("wd", [EPC, KH, 128, KI, 128], F32R,
                        kind="ExternalInput")
    cw = nc.dram_tensor("cw", [EPC, C], F32, kind="ExternalInput")
    yg = nc.dram_tensor("yg", [EPC, 128, KH, C], F32, kind="ExternalOutput")
    warm_out = nc.dram_tensor("warm", [128, 1], F32, kind="ExternalOutput")

    cts = _ctiles(C)
    with ExitStack() as ctx:
        tc = ctx.enter_context(tile.TileContext(nc))
        cwp = ctx.enter_context(tc.tile_pool(name="cwp", bufs=1))
        xgp = ctx.enter_context(tc.tile_pool(name="xgp", bufs=2))
        wgup = ctx.enter_context(tc.tile_pool(name="wgup", bufs=6))
        wdp = ctx.enter_context(tc.tile_pool(name="wdp", bufs=4))
        hp = ctx.enter_context(tc.tile_pool(name="hp", bufs=2))
        msc = ctx.enter_context(tc.tile_pool(name="msc", bufs=4))
        outp = ctx.enter_context(tc.tile_pool(name="outp", bufs=2))
        ps_gu = ctx.enter_context(tc.tile_pool(name="ps_gu", bufs=2, space="PSUM"))
        ps_d = ctx.enter_context(tc.tile_pool(name="ps_d", bufs=2, space="PSUM"))

        cwb = []
        for j in range(EPC):
            cwt = cwp.tile([128, C], F32, tag=f"cw{j}")
            nc.gpsimd.dma_start(out=cwt, in_=cw[j:j + 1, :].partition_broadcast(128))
            cwb.append(cwt)

        # PE warm-up: keep TensorE busy while the first weight/activation
        # DMAs land, so the HAM clock-gate releases (1.2 -> 2.4 GHz) before
        # real matmuls start. Results are dumped to a debug output.
        wtile = cwp.tile([128, 512], F32R, tag="warm")
        nc.vector.memset(wtile.bitcast(F32), 0.0)
        wps = ps_d.tile([128, 512], F32, tag="warmp")
        for wi in range(6):
            nc.tensor.matmul(wps, wtile[:, :128], wtile,
                             start=(wi == 0), stop=(wi == 5))
        wres = cwp.tile([128, 1], F32, tag="warmres")
        nc.vector.tensor_copy(wres, wps[:, 0:1])
        nc.gpsimd.dma_start(out=warm_out[:], in_=wres)

        for _ in range(reps):
            for j in range(EPC):
                xg_t = xgp.tile([128, KH, C], F32R)
                for k in range(KH):
                    eng = nc.sync if k % 2 == 0 else nc.scalar
                    eng.dma_start(out=xg_t[:, k, :], in_=xg[j, :, k, :])
                h_t = hp.tile([128, KI, C], F32R)
                for m in range(KI):
                    wgu_t = wgup.tile([128, 2, KH, 128], F32R)
                    nc.sync.dma_start(out=wgu_t[:, 0], in_=wgu[j, m, 0])
                    nc.scalar.dma_start(out=wgu_t[:, 1], in_=wgu[j, m, 1])
                    for (c0, cn) in cts:
                        gp = ps_gu.tile([128, cn], F32, tag="gp")
                        for k in range(KH):
                            nc.tensor.matmul(gp, wgu_t[:, 0, k, :],
                                             xg_t[:, k, c0:c0 + cn],
                                             start=(k == 0), stop=(k == KH - 1))
                        up = ps_gu.tile([128, cn], F32, tag="up")
                        for k in range(KH):
                            nc.tensor.matmul(up, wgu_t[:, 1, k, :],
                                             xg_t[:, k, c0:c0 + cn],
                                             start=(k == 0), stop=(k == KH - 1))
                        sg = msc.tile([128, cn], F32, tag="sg")
                        nc.scalar.activation(sg, gp,
                                             mybir.ActivationFunctionType.Silu)
                        t1 = msc.tile([128, cn], F32, tag="t1")
                        nc.vector.tensor_mul(t1, sg, up)
                        nc.vector.tensor_mul(h_t[:, m, c0:c0 + cn], t1,
                                             cwb[j][:, c0:c0 + cn])
                yo_all = outp.tile([128, KH, C], F32, tag="yo")
                for h in range(KH):
                    wd_t = wdp.tile([128, KI, 128], F32R)
                    eng = nc.sync if h % 2 == 0 else nc.scalar
                    eng.dma_start(out=wd_t, in_=wd[j, h])
                    for (c0, cn) in cts:
                        yp = ps_d.tile([128, cn], F32, tag="yp")
                        for k in range(KI):
                            nc.tensor.matmul(yp, wd_t[:, k, :],
                                             h_t[:, k, c0:c0 + cn],
                                             start=(k == 0), stop=(k == KI - 1))
                        nc.vector.tensor_copy(yo_all[:, h, c0:c0 + cn], yp)
                    if h % 2 == 1:
                        # stream out in 2-h chunks as they complete (SWDGE,
                        # keeping both HWDGE engines free for input reads)
                        nc.gpsimd.dma_start(out=yg[j, :, h - 1:h + 1, :],
                                            in_=yo_all[:, h - 1:h + 1, :])
    nc.finalize()
    return nc


def _get_router():
    if "router" not in _program_cache:
        _program_cache["router"] = build_router()
    return _program_cache["router"]


def _get_experts(C):
    key = ("experts", C)
    if key not in _program_cache:
        _program_cache[key] = build_experts(C)
    return _program_cache[key]


def prep_router_inputs(x):
    xT = np.ascontiguousarray(x.T)
    return xT


def route_on_host(combine):
    idx = [np.nonzero(combine[:, e])[0] for e in range(E)]
    maxn = max(len(ii) for ii in idx)
    C = max(512, ((maxn + 127) // 128) * 128)
    return idx, C


def prep_expert_inputs(x, combine, idx, C, w_gate, w_up, w_down):
    """Build per-core in_maps with tile-exact layouts (all contiguous DMA)."""
    in_maps = []
    for c in range(NCORES):
        xg = np.zeros((EPC, 128, KH, C), np.float32)
        cwm = np.zeros((EPC, C), np.float32)
        wgu = np.empty((EPC, KI, 2, 128, KH, 128), np.float32)
        wdh = np.empty((EPC, KH, 128, KI, 128), np.float32)
        for j in range(EPC):
            e = c * EPC + j
            ii = idx[e]
            n = len(ii)
            if n:
                # [n, H] -> [n, KH, 128] -> [128, KH, n]
                xe = x[ii].reshape(n, KH, 128).transpose(2, 1, 0)
                xg[j, :, :, :n] = xe
                cwm[j, :n] = combine[ii, e]
            g = w_gate[e].reshape(KI, 128, KH, 128)   # (m, i, k, p)
            u = w_up[e].reshape(KI, 128, KH, 128)
            wgu[j, :, 0] = g.transpose(0, 3, 2, 1)    # (m, p, k, i)
            wgu[j, :, 1] = u.transpose(0, 3, 2, 1)
            d = w_down[e].reshape(KH, 128, KI, 128)   # (h, o, k, p)
            wdh[j] = d.transpose(0, 3, 2, 1)          # (h, p, k, o)
        in_maps.append({"xg": xg, "wgu": wgu, "wd": wdh, "cw": cwm})
    return in_maps


def kernel(hidden_states, w_router, w_gate, w_up, w_down):
    x = np.ascontiguousarray(np.asarray(hidden_states, np.float32)).reshape(T, H)
    w_gate = np.asarray(w_gate, np.float32)
    w_up = np.asarray(w_up, np.float32)
    w_down = np.asarray(w_down, np.float32)
    xT = prep_router_inputs(x)
    wrT = np.ascontiguousarray(np.asarray(w_router, np.float32).T)   # [H, E]

    # ---- Phase 1: router on device ----
    nc1 = _get_router()
    in_maps1 = [
        {"xTc": np.ascontiguousarray(xT[:, c * TPC:(c + 1) * TPC]), "wrT": wrT}
        for c in range(NCORES)
    ]
    r1 = run_bass_kernel_spmd(nc1, in_maps1, list(range(NCORES)))
    combine = np.concatenate(
        [r1.results[c]["comb"].reshape(TPC, E) for c in range(NCORES)], axis=0)

    # ---- Host: compaction (data movement only) ----
    idx, C = route_on_host(combine)
    in_maps2 = prep_expert_inputs(x, combine, idx, C, w_gate, w_up, w_down)

    # ---- Phase 2: expert MLPs on device ----
    nc2 = _get_experts(C)
    r2 = run_bass_kernel_spmd(nc2, in_maps2, list(range(NCORES)))

    # ---- Host: scatter-add (unique indices per expert) ----
    y = np.zeros((T, H), np.float32)
    for c in range(NCORES):
        ygc = r2.results[c]["yg"]          # [EPC, 128, KH, C]
        for j in range(EPC):
            e = c * EPC + j
            ii = idx[e]
            n = len(ii)
            if n:
                # [128(p), KH(h), C] -> [H, C]: H index = h*128 + p
                yt = ygc[j].transpose(1, 0, 2).reshape(H, C)
                y[ii] += yt[:, :n].T
    return y.reshape(B, S, H)



# revision 4
# speedup vs baseline: 4.3421x; 4.3421x over previous
"""MoE (MiMoV2 FlashMoE) Trainium2 kernel: expert-parallel over 8 NeuronCores.

Strategy:
  Phase 1 (device): router — logits = x @ w_router.T computed in exact fp32
    (selection must match the reference bit-for-bit; any top-4 flip costs
    ~14% output error), top-4 via iterative max/mask, combine weights =
    sigmoid(logit) normalized over the selected 4. Each core handles
    T/8 = 512 tokens. Output: dense combine matrix [T, E].
  Host: compaction — per-expert token index lists (data movement only),
    load-balanced expert->(core,slot) assignment: experts sorted by load
    descending, slot j takes ranks [8j, 8j+8) one per core, slot capacity
    = max load in the slot (rounded up to 32). This cuts padded columns
    ~16% vs a global fixed capacity.
  Phase 2 (device): experts — one expert per (core, slot). All matmuls in
    bf16 (same 1 cycle/row PE rate as f32r on TRN2, half the HBM/SBUF
    traffic; output rel err ~3e-3 vs the 2e-2 budget). G = Wg x, U = Wu x
    accumulate in fp32 PSUM; H = silu(G)*U*combine in fp32, cast to bf16;
    Y = Wd H in fp32 out.
  Host: scatter-add per-expert outputs into y [T, H].
"""
import math
import numpy as np
import ml_dtypes
from contextlib import ExitStack

import concourse.bass as bass
import concourse.mybir as mybir
import concourse.tile as tile
from concourse import bacc
from concourse.bass_utils import run_bass_kernel_spmd

F32 = mybir.dt.float32
F32R = mybir.dt.float32r
BF16 = mybir.dt.bfloat16
NPBF16 = ml_dtypes.bfloat16

# Problem shapes (hardcoded per contract)
E = 32          # experts
TOPK = 4
H = 1024        # hidden
I = 768         # intermediate
B, S = 2, 2048
T = B * S       # 4096 tokens
NCORES = 8
EPC = E // NCORES    # expert slots per core = 4
TPC = T // NCORES    # router tokens per core = 512
KH = H // 128        # 8 contraction chunks over H
KI = I // 128        # 6 contraction chunks over I

_program_cache = {}


def _ctiles(C):
    """Split C into near-equal tiles, each <= 512 (PSUM bank)."""
    n = max(1, math.ceil(C / 512))
    base = C // n
    rem = C - base * n
    sizes = [base + (1 if i < rem else 0) for i in range(n)]
    out, off = [], 0
    for s in sizes:
        out.append((off, s))
        off += s
    return out


def build_router(reps=1):
    """Per-core: logits^T = w_router @ x^T via PE (weights stationary, 512
    tokens moving), DVE 32x32 block transposes to [tokens, E], then a
    batched top-4 + combine-weight computation on a single [128, 4, E]
    tile. Selection compares exact fp32 logits."""
    nc = bacc.Bacc()
    NT = TPC // 128  # 4 token tiles
    xTc = nc.dram_tensor("xTc", [H, TPC], F32, kind="ExternalInput")
    wrT = nc.dram_tensor("wrT", [H, E], F32, kind="ExternalInput")
    comb_out = nc.dram_tensor("comb", [NT, 128, E], F32, kind="ExternalOutput")
    with ExitStack() as ctx:
        tc = ctx.enter_context(tile.TileContext(nc))
        sb = ctx.enter_context(tc.tile_pool(name="sb", bufs=1))
        work = ctx.enter_context(tc.tile_pool(name="work", bufs=2))
        ps = ctx.enter_context(tc.tile_pool(name="ps", bufs=2, space="PSUM"))

        xr = sb.tile([128, KH, TPC], F32)
        wr = sb.tile([128, KH, E], F32)
        for k in range(KH):
            nc.sync.dma_start(out=xr[:, k, :], in_=xTc[k * 128:(k + 1) * 128, :])
            nc.sync.dma_start(out=wr[:, k, :], in_=wrT[k * 128:(k + 1) * 128, :])

        for _ in range(reps):
            lgT_p = ps.tile([E, TPC], F32)   # logits^T, 1 PSUM bank
            for k in range(KH):
                nc.tensor.matmul(lgT_p, wr[:, k, :], xr[:, k, :],
                                 start=(k == 0), stop=(k == KH - 1))
            lgT = work.tile([E, TPC], F32)
            nc.vector.tensor_copy(lgT, lgT_p)
            # transpose to [128, NT, E] with DVE 32x32 block transposes
            lt = work.tile([128, NT, E], F32)
            for t in range(NT):
                for jb in range(128 // 32):
                    nc.vector.transpose(
                        lt[jb * 32:(jb + 1) * 32, t, :],
                        lgT[:, t * 128 + jb * 32: t * 128 + (jb + 1) * 32])
            # batched top-4: find 4th max per token via iterative masking
            cur = work.tile([128, NT, E], F32)
            nc.vector.tensor_copy(cur, lt)
            m = work.tile([128, NT, 1], F32)
            ge = work.tile([128, NT, E], F32)
            for _k in range(TOPK - 1):
                nc.vector.reduce_max(m, cur, axis=mybir.AxisListType.X)
                nc.vector.tensor_tensor(ge, cur, m.broadcast_to((128, NT, E)),
                                        op=mybir.AluOpType.is_ge)
                nc.vector.scalar_tensor_tensor(cur, ge, -1e30, cur,
                                               op0=mybir.AluOpType.mult,
                                               op1=mybir.AluOpType.add)
            nc.vector.reduce_max(m, cur, axis=mybir.AxisListType.X)
            # sel = (logits >= 4th max), combine = sel*sigmoid normalized
            sel = work.tile([128, NT, E], F32)
            nc.vector.tensor_tensor(sel, lt, m.broadcast_to((128, NT, E)),
                                    op=mybir.AluOpType.is_ge)
            sig = work.tile([128, NT, E], F32)
            nc.scalar.activation(sig, lt, mybir.ActivationFunctionType.Sigmoid)
            wsel = work.tile([128, NT, E], F32)
            nc.vector.tensor_mul(wsel, sel, sig)
            ssum = work.tile([128, NT, 1], F32)
            nc.vector.reduce_sum(ssum, wsel, axis=mybir.AxisListType.X)
            nc.vector.tensor_scalar_add(ssum, ssum, 1e-20)
            rsum = work.tile([128, NT, 1], F32)
            nc.vector.reciprocal(rsum, ssum)
            ct = work.tile([128, NT, E], F32)
            nc.vector.tensor_tensor(ct, wsel, rsum.broadcast_to((128, NT, E)),
                                    op=mybir.AluOpType.mult)
            for t in range(NT):
                nc.sync.dma_start(out=comb_out[t], in_=ct[:, t, :])
    nc.finalize()
    return nc


def build_experts(caps, reps=1):
    """Expert MLP kernel, one expert per slot j with capacity caps[j].
    Per-core inputs (pre-laid-out for SBUF tiles, all bf16 except cw):
      xg{j} [128, KH, caps[j]]        bf16  xg[p,k,c] = x[tok_c, k*128+p]
      wgu   [EPC, KI, 2, 128, KH, 128] bf16 [...,0,i]=w_gate[e,m*128+i,k*128+p]
      wd    [EPC, KH, 128, KI, 128]   bf16  wd[j,h,p,k,o]=w_down[e,h*128+o,k*128+p]
      cw{j} [1, caps[j]]              f32   combine weights (0 on padding)
    Output: yg{j} [128, KH, caps[j]] f32, yg[p,h,c] = y^T[h*128+p, c]
    (combine-weighted, transposed)."""
    caps = tuple(caps)
    nc = bacc.Bacc()
    xg = [nc.dram_tensor(f"xg{j}", [128, KH, caps[j]], BF16,
                         kind="ExternalInput") for j in range(EPC)]
    wgu = nc.dram_tensor("wgu", [EPC, KI, 2, 128, KH, 128], BF16,
                         kind="ExternalInput")
    wd = nc.dram_tensor("wd", [EPC, KH, 128, KI, 128], BF16,
                        kind="ExternalInput")
    cw = [nc.dram_tensor(f"cw{j}", [1, caps[j]], F32, kind="ExternalInput")
          for j in range(EPC)]
    yg = [nc.dram_tensor(f"yg{j}", [128, KH, caps[j]], F32,
                         kind="ExternalOutput") for j in range(EPC)]
    warm_out = nc.dram_tensor("warm", [128, 1], F32, kind="ExternalOutput")

    with ExitStack() as ctx:
        tc = ctx.enter_context(tile.TileContext(nc))
        cwp = ctx.enter_context(tc.tile_pool(name="cwp", bufs=1))
        xgp = ctx.enter_context(tc.tile_pool(name="xgp", bufs=2))
        wgup = ctx.enter_context(tc.tile_pool(name="wgup", bufs=6))
        wdp = ctx.enter_context(tc.tile_pool(name="wdp", bufs=4))
        hp = ctx.enter_context(tc.tile_pool(name="hp", bufs=2))
        msc = ctx.enter_context(tc.tile_pool(name="msc", bufs=4))
        outp = ctx.enter_context(tc.tile_pool(name="outp", bufs=2))
        ps_gu = ctx.enter_context(tc.tile_pool(name="ps_gu", bufs=2, space="PSUM"))
        ps_d = ctx.enter_context(tc.tile_pool(name="ps_d", bufs=2, space="PSUM"))

        cwb = []
        for j in range(EPC):
            cwt = cwp.tile([128, caps[j]], F32, tag=f"cw{j}")
            nc.gpsimd.dma_start(out=cwt,
                                in_=cw[j][0:1, :].partition_broadcast(128))
            cwb.append(cwt)

        # PE warm-up: keep TensorE busy while the first weight/activation
        # DMAs land, so the HAM clock-gate releases (1.2 -> 2.4 GHz) before
        # real matmuls start. Results are dumped to a debug output.
        wtile = cwp.tile([128, 512], F32R, tag="warm")
        nc.vector.memset(wtile.bitcast(F32), 0.0)
        wps = ps_d.tile([128, 512], F32, tag="warmp")
        for wi in range(6):
            nc.tensor.matmul(wps, wtile[:, :128], wtile,
                             start=(wi == 0), stop=(wi == 5))
        wres = cwp.tile([128, 1], F32, tag="warmres")
        nc.vector.tensor_copy(wres, wps[:, 0:1])
        nc.gpsimd.dma_start(out=warm_out[:], in_=wres)

        for _ in range(reps):
            for j in range(EPC):
                cap = caps[j]
                cts = _ctiles(cap)
                xg_t = xgp.tile([128, KH, cap], BF16)
                for k in range(KH):
                    eng = nc.sync if k % 2 == 0 else nc.scalar
                    eng.dma_start(out=xg_t[:, k, :], in_=xg[j][:, k, :])
                h_t = hp.tile([128, KI, cap], BF16)
                for m in range(KI):
                    wgu_t = wgup.tile([128, 2, KH, 128], BF16)
                    nc.sync.dma_start(out=wgu_t[:, 0], in_=wgu[j, m, 0])
                    nc.scalar.dma_start(out=wgu_t[:, 1], in_=wgu[j, m, 1])
                    for (c0, cn) in cts:
                        gp = ps_gu.tile([128, cn], F32, tag="gp")
                        for k in range(KH):
                            nc.tensor.matmul(gp, wgu_t[:, 0, k, :],
                                             xg_t[:, k, c0:c0 + cn],
                                             start=(k == 0), stop=(k == KH - 1))
                        up = ps_gu.tile([128, cn], F32, tag="up")
                        for k in range(KH):
                            nc.tensor.matmul(up, wgu_t[:, 1, k, :],
                                             xg_t[:, k, c0:c0 + cn],
                                             start=(k == 0), stop=(k == KH - 1))
                        sg = msc.tile([128, cn], F32, tag="sg")
                        nc.scalar.activation(sg, gp,
                                             mybir.ActivationFunctionType.Silu)
                        t1 = msc.tile([128, cn], F32, tag="t1")
                        nc.vector.tensor_mul(t1, sg, up)
                        nc.vector.tensor_mul(h_t[:, m, c0:c0 + cn], t1,
                                             cwb[j][:, c0:c0 + cn])
                yo_all = outp.tile([128, KH, cap], F32, tag="yo")
                for h in range(KH):
                    wd_t = wdp.tile([128, KI, 128], BF16)
                    eng = nc.sync if h % 2 == 0 else nc.scalar
                    eng.dma_start(out=wd_t, in_=wd[j, h])
                    for (c0, cn) in cts:
                        yp = ps_d.tile([128, cn], F32, tag="yp")
                        for k in range(KI):
                            nc.tensor.matmul(yp, wd_t[:, k, :],
                                             h_t[:, k, c0:c0 + cn],
                                             start=(k == 0), stop=(k == KI - 1))
                        nc.vector.tensor_copy(yo_all[:, h, c0:c0 + cn], yp)
                    if h % 2 == 1:
                        # stream out in 2-h chunks as they complete (SWDGE,
                        # keeping both HWDGE engines free for input reads)
                        nc.gpsimd.dma_start(out=yg[j][:, h - 1:h + 1, :],
                                            in_=yo_all[:, h - 1:h + 1, :])
    nc.finalize()
    return nc


def _get_router():
    if "router" not in _program_cache:
        _program_cache["router"] = build_router()
    return _program_cache["router"]


def _get_experts(caps):
    key = ("experts", tuple(caps))
    if key not in _program_cache:
        _program_cache[key] = build_experts(caps)
    return _program_cache[key]


def prep_router_inputs(x):
    xT = np.ascontiguousarray(x.T)
    return xT


def plan_experts(combine):
    """Load-balanced assignment: experts sorted by load descending; slot j
    takes ranks [8j, 8j+8), one per core; cap_j = max load in slot j
    (rounded up to 32). Returns per-expert index lists, assignment
    (core, slot) -> expert, and slot capacities."""
    loads = (combine > 0).sum(axis=0).astype(np.int64)
    order = np.argsort(-loads, kind="stable")
    assign = [[int(order[8 * j + c]) for j in range(EPC)]
              for c in range(NCORES)]
    caps = tuple(int(max(64, math.ceil(loads[order[8 * j]] / 32) * 32))
                 for j in range(EPC))
    idx = [np.nonzero(combine[:, e])[0] for e in range(E)]
    return idx, assign, caps


def prep_expert_weights(w_gate, w_up, w_down):
    """Per-expert bf16 tile-exact layouts (done once per kernel() call)."""
    gb = w_gate.astype(NPBF16).reshape(E, KI, 128, KH, 128)
    ub = w_up.astype(NPBF16).reshape(E, KI, 128, KH, 128)
    db = w_down.astype(NPBF16).reshape(E, KH, 128, KI, 128)
    gb = np.ascontiguousarray(gb.transpose(0, 1, 4, 3, 2))  # (e, m, p, k, i)
    ub = np.ascontiguousarray(ub.transpose(0, 1, 4, 3, 2))
    db = np.ascontiguousarray(db.transpose(0, 1, 4, 3, 2))  # (e, h, p, k, o)
    return gb, ub, db


def prep_expert_inputs(x, combine, idx, assign, caps, gb, ub, db):
    """Build per-core in_maps with tile-exact layouts (contiguous DMA)."""
    xb = x.astype(NPBF16)
    in_maps = []
    for c in range(NCORES):
        m = {}
        wgu = np.empty((EPC, KI, 2, 128, KH, 128), NPBF16)
        wdh = np.empty((EPC, KH, 128, KI, 128), NPBF16)
        for j in range(EPC):
            e = assign[c][j]
            ii = idx[e]
            n = len(ii)
            cap = caps[j]
            xgj = np.zeros((128, KH, cap), NPBF16)
            cwj = np.zeros((1, cap), np.float32)
            if n:
                # [n, H] -> [n, KH, 128] -> [128, KH, n]
                xgj[:, :, :n] = xb[ii].reshape(n, KH, 128).transpose(2, 1, 0)
                cwj[0, :n] = combine[ii, e]
            m[f"xg{j}"] = xgj
            m[f"cw{j}"] = cwj
            wgu[j, :, 0] = gb[e]
            wgu[j, :, 1] = ub[e]
            wdh[j] = db[e]
        m["wgu"] = wgu
        m["wd"] = wdh
        in_maps.append(m)
    return in_maps


def kernel(hidden_states, w_router, w_gate, w_up, w_down):
    x = np.ascontiguousarray(np.asarray(hidden_states, np.float32)).reshape(T, H)
    w_gate = np.asarray(w_gate, np.float32)
    w_up = np.asarray(w_up, np.float32)
    w_down = np.asarray(w_down, np.float32)
    xT = prep_router_inputs(x)
    wrT = np.ascontiguousarray(np.asarray(w_router, np.float32).T)   # [H, E]

    # ---- Phase 1: router on device ----
    nc1 = _get_router()
    in_maps1 = [
        {"xTc": np.ascontiguousarray(xT[:, c * TPC:(c + 1) * TPC]), "wrT": wrT}
        for c in range(NCORES)
    ]
    r1 = run_bass_kernel_spmd(nc1, in_maps1, list(range(NCORES)))
    combine = np.concatenate(
        [r1.results[c]["comb"].reshape(TPC, E) for c in range(NCORES)], axis=0)

    # ---- Host: compaction (data movement only) ----
    idx, assign, caps = plan_experts(combine)
    gb, ub, db = prep_expert_weights(w_gate, w_up, w_down)
    in_maps2 = prep_expert_inputs(x, combine, idx, assign, caps, gb, ub, db)

    # ---- Phase 2: expert MLPs on device ----
    nc2 = _get_experts(caps)
    r2 = run_bass_kernel_spmd(nc2, in_maps2, list(range(NCORES)))

    # ---- Host: scatter-add (unique indices per expert) ----
    y = np.zeros((T, H), np.float32)
    for c in range(NCORES):
        for j in range(EPC):
            e = assign[c][j]
            ii = idx[e]
            n = len(ii)
            if n:
                # [128(p), KH(h), cap] -> [H, cap]: H index = h*128 + p
                ygj = r2.results[c][f"yg{j}"]
                yt = ygj.transpose(1, 0, 2).reshape(H, caps[j])
                y[ii] += yt[:, :n].T
    return y.reshape(B, S, H)
